# revision 24
# baseline (speedup 1.0000x reference)
"""Trainium2 Bass kernel for a dense transformer encoder layer.

Model (faithful to the oracle):
  q,k,v = x@wq+bq, x@wk+bk, x@wv+bv          (12 heads, dk=64, DIM=768)
  scores = q@k^T / sqrt(768)  (note: sqrt(dim_model), not sqrt(dk))
  scores[mask==0] = 1e-11  (NOT -inf; masked keys still contribute ~1/Z)
  attn = softmax(scores); z = attn@v; o = z@wo+bo
  l1 = x + LN(o);  ffn = relu(l1@w1+b1)@w2+b2;  out = l1 + LN(ffn)

Sharding: 4096 tokens (B=2,S=2048) split 8 ways -> 512 tokens/core.
Cores 0-3 own batch 0, cores 4-7 batch 1. No collectives: each core
computes K/V itself — but only for the UNMASKED keys of its batch.

Mask compaction: masked keys (~half) all get score 1e-11, i.e. exp==1
(fp32), for every query/head. Their attention contribution is a
query-independent constant: C_h = sum_masked v_k (numerator) and
m = #masked (denominator). The host compacts unmasked tokens into a
padded [KC] buffer, computes the tiny C correction in numpy, and the
device runs attention only over the compacted keys, seeding the attn@v
PSUM accumulation with C via a K=1 bf16 matmul. Padded key slots have
K=V=0 exactly (biases are added inside the matmuls via a 0/1 validity
row), so exp(score)=1 * V=0 contributes nothing; the validity row also
zeroes their denominator entry.

Everything stays bf16: fp8 anywhere in the attention path (q/k/v
projections, stored V, exp weights) measurably costs ~1.3e-2 relative
error each — the attention output is a near-uniform average over ~2k
keys, so its *signal* is small and per-element quantization noise does
NOT average away relative to it.

Perf changes vs the 256.9us v2 baseline (now ~219us):
 - Q projection is kt-outer over 6 PSUM banks and the x^T/wq DMAs are
   interleaved per 128-row chunk, so the first matmul starts ~10us in
   instead of ~23us; bq is added on the ACT PSUM->SBUF copy
   (per-partition bias) instead of a PE matmul.
 - V projection runs kt-outer in 3 passes of <=8 PSUM banks so it can
   chew each (xkv,wv) chunk as it lands instead of waiting for all six.
 - The C-seed matmuls are bf16 (were fp32 = 4 cycles/row on the PE).
 - z^T lives in one tile per head-pair, so the O-projection's kt<5
   matmuls start as soon as the last head-pair's attn@v PE work ends
   instead of serializing behind its softmax tail; the kt=5 finishers
   trail two token-tiles behind the kt0-4 accumulations.
 - attn@v emits each half's softmax tail (reciprocal + gpsimd
   partition-broadcast) immediately, overlapping the other half.
 - Phases 3-5 are merged into one per-token-tile pipeline
   (O-proj -> LN1 -> transpose -> FFN1 -> FFN2 -> LN2 -> store) with
   relu+bias on the ACT engine (per-partition b1); LN row-sums come
   from accum_out on the producers' bias-add STTs; transposes for
   tile tt+1 are issued before FFN2(tt).
 - l1 is kept in bf16 (residual quantization ~0.2%), making the
   transposes 1 cycle/row instead of 2 (fp32).
 - All of w2 is prefetched (12 tiles behind w1 on the sync queue, 12
   right after attention) so FFN2 of token tile 0 never waits.
 - When bk/bv are all-zero (host-checked), a program variant skips the
   K/V bias matmuls; the general variant compiles on demand otherwise.

Further (this file): FFN1 runs as 24 N=512 groups off a full-width
l1^T (the ~200ns per-matmul-GROUP boundary tax made 96 N=128 groups
~14us slower); LN row-sums via accum_out, x^2 sums via ACT
Square+accum_out; when biases AND LayerNorm affines are trivial
(host-checked, the general variant compiles on demand), the K/V bias
matmuls and affine/residual-bias ops are skipped and the normalize
collapses to one ACT op (per-partition scale+bias) + one add; the LN
pools run bufs=4 and the chains are issued AFTER all O-proj
STTs so their ACT<->DVE ping-pong never blocks the DVE queue head.

Measured-but-reverted (regressions): fp8 anywhere in attention
(error), bootstrapping DMAs on the gpsimd queue (that ring starts
LATER than sync for bulk), bf16 partition-broadcasts (slower), FFN1
at N=256 token-tile pairs (deepens the LN1->transpose barrier),
ps_o bufs=4 (no effect on the DVE-serialized O epilogue).
"""

import math
import os
import sys

import numpy as np

for _p in ("/opt/trn_rl_repo", os.path.expanduser("~/.axon_site/_ro/trn_rl_repo")):
    if os.path.isdir(_p) and _p not in sys.path:
        sys.path.insert(0, _p)

import ml_dtypes  # noqa: E402

BF16 = ml_dtypes.bfloat16

DIM = 768
HEADS = 12
DK = 64
HID = 4 * DIM  # 3072
B, S = 2, 2048
N_CORES = 8
BLK = 512            # tokens per core
NBLK = S // BLK      # 4 blocks per batch
KC = 1152            # compacted-key capacity (9 tiles; ~5.7 sigma above E[n_u])
EPS = 1e-5
ISCALE = 1.0 / math.sqrt(DIM)

FT = DIM // 128   # 6 feature tiles
TT = BLK // 128   # 4 token tiles per core block
STC = KC // 128   # 9 compacted key tiles
HT = HID // 128   # 24 hidden tiles
W2A = 12          # w2 tiles prefetched during attention

_CACHE: dict = {}


def _build_program(general: bool):
    import concourse.bass as bass
    import concourse.mybir as mybir
    import concourse.tile as tile
    from concourse import bacc
    from concourse.masks import make_identity

    f32 = mybir.dt.float32
    bf16 = mybir.dt.bfloat16
    AF = mybir.ActivationFunctionType
    ALU = mybir.AluOpType
    AX = mybir.AxisListType

    nc = bacc.Bacc()

    # ---- per-core DRAM I/O ----
    d_xTb = nc.dram_tensor("xTb", [DIM, BLK], bf16, kind="ExternalInput")
    d_xb = nc.dram_tensor("xb", [BLK, DIM], f32, kind="ExternalInput")
    d_xkvT = nc.dram_tensor("xkvT", [DIM, KC], bf16, kind="ExternalInput")
    d_onesc = nc.dram_tensor("onesc", [KC], bf16, kind="ExternalInput")
    d_wq = nc.dram_tensor("wq", [DIM, DIM], bf16, kind="ExternalInput")
    d_wk = nc.dram_tensor("wk", [DIM, DIM], bf16, kind="ExternalInput")
    d_wv = nc.dram_tensor("wv", [DIM, DIM], bf16, kind="ExternalInput")
    d_wo = nc.dram_tensor("wo", [DIM, DIM], bf16, kind="ExternalInput")
    d_w1 = nc.dram_tensor("w1", [DIM, HID], bf16, kind="ExternalInput")
    d_w2 = nc.dram_tensor("w2", [HID, DIM], bf16, kind="ExternalInput")
    d_bkrow = nc.dram_tensor("bkrow", [DIM], bf16, kind="ExternalInput")
    d_bvrow = nc.dram_tensor("bvrow", [DIM], bf16, kind="ExternalInput")
    d_bqT = nc.dram_tensor("bqT", [DIM], f32, kind="ExternalInput")
    d_crow = nc.dram_tensor("crow", [HEADS * (DK + 1)], bf16, kind="ExternalInput")
    d_bo = nc.dram_tensor("bo", [DIM], f32, kind="ExternalInput")
    d_b1 = nc.dram_tensor("b1", [HID], f32, kind="ExternalInput")
    d_b2 = nc.dram_tensor("b2", [DIM], f32, kind="ExternalInput")
    d_g1 = nc.dram_tensor("g1", [DIM], f32, kind="ExternalInput")
    d_bb1 = nc.dram_tensor("bb1", [DIM], f32, kind="ExternalInput")
    d_g2 = nc.dram_tensor("g2", [DIM], f32, kind="ExternalInput")
    d_bb2 = nc.dram_tensor("bb2", [DIM], f32, kind="ExternalInput")
    d_out = nc.dram_tensor("out", [BLK, DIM], f32, kind="ExternalOutput")

    KCH = [(0, 512), (512, 1024), (1024, KC)]  # K-proj N chunks
    # V kt-outer passes: (nh, tt) groups, <=8 PSUM banks at a time
    VGRP = [(nh, tt) for nh in range(2) for tt in range(STC)]
    VPASS = [VGRP[0:8], VGRP[8:16], VGRP[16:18]]

    def bcast_ap(handle, n=128):
        ap = handle[:]
        return bass.AP(tensor=ap.tensor, offset=ap.offset, ap=[[0, n]] + list(ap.ap))

    with tile.TileContext(nc) as tc:
        with (
            tc.tile_pool(name="const", bufs=1) as const,
            tc.tile_pool(name="bigres", bufs=1) as big,
        ):
            # ---------- small constants (gpsimd queue) ----------
            sb_b1 = const.tile([128, HT], f32)
            nc.gpsimd.dma_start(
                out=sb_b1, in_=d_b1[:].rearrange("(t p) -> p t", p=128)
            )
            sb_bqT = const.tile([128, FT], f32)
            nc.gpsimd.dma_start(
                out=sb_bqT, in_=d_bqT[:].rearrange("(t p) -> p t", p=128)
            )
            sb_bkrow = const.tile([1, DIM], bf16)
            nc.gpsimd.dma_start(out=sb_bkrow, in_=bcast_ap(d_bkrow, 1))
            sb_bvrow = const.tile([1, DIM], bf16)
            nc.gpsimd.dma_start(out=sb_bvrow, in_=bcast_ap(d_bvrow, 1))
            sb_crow = const.tile([1, HEADS * (DK + 1)], bf16)
            nc.gpsimd.dma_start(out=sb_crow, in_=bcast_ap(d_crow, 1))
            sb_onesc = const.tile([1, KC], bf16)
            nc.gpsimd.dma_start(out=sb_onesc, in_=bcast_ap(d_onesc, 1))
            sb_onescT = const.tile([128, STC], bf16)
            nc.gpsimd.dma_start(
                out=sb_onescT, in_=d_onesc[:].rearrange("(t p) -> p t", p=128)
            )
            ones512b = const.tile([1, BLK], bf16)
            nc.vector.memset(ones512b, 1.0)
            eps_t = const.tile([128, 1], f32)
            nc.vector.memset(eps_t, EPS)

            # ---------- persistent activations ----------
            sb_xblk = big.tile([128, TT, DIM], f32)  # residual x
            sb_l1 = big.tile([128, TT, DIM], bf16)

            # FFN/O weights: loaded behind the attention inputs on the sync
            # queue, consumed post-attention
            wpre_cm = tc.tile_pool(name="wpre", bufs=1)
            wpre = wpre_cm.__enter__()
            w_o = wpre.tile([128, FT, DIM], bf16)
            w1_t = []
            for kt in range(FT):
                w1_kt = wpre.tile([128, HID], bf16, tag=f"w1_{kt}", name=f"w1_{kt}")
                w1_t.append(w1_kt)
            w2_t = []
            for kt in range(W2A):
                w2_kt = wpre.tile([128, DIM], bf16, tag=f"w2_{kt}", name=f"w2_{kt}")
                w2_t.append(w2_kt)

            # z^T outlives the K/Q/V residents (consumed by the O-proj).
            # One tile per head-pair so O-proj reads of early head-pairs
            # don't serialize behind the last head-pair's softmax tail.
            zT_cm = tc.tile_pool(name="zTp", bufs=1)
            zTp = zT_cm.__enter__()
            sb_zT = [
                zTp.tile([128, BLK], bf16, tag=f"zT{hp}", name=f"zT{hp}")
                for hp in range(HEADS // 2)
            ]

            kqv_cm = tc.tile_pool(name="kqvp", bufs=1)
            kqvp = kqv_cm.__enter__()
            sb_K = kqvp.tile([128, FT, KC], bf16)  # K^T, feat-major
            sb_Q = kqvp.tile([128, FT, BLK], bf16)  # Q^T, feat-major
            sb_V = kqvp.tile([128, STC, HEADS, DK + 1], bf16)  # V + ones col

            # ===== Phase 1+2: QKV projections fused with attention =====
            with (
                tc.tile_pool(name="xw_b", bufs=1) as xwb,
                tc.tile_pool(name="xw_a", bufs=1) as xwa,
                tc.tile_pool(name="expp", bufs=20) as expp,
                tc.tile_pool(name="attsm", bufs=1) as attsm,
            ):
                sb_xkvT = xwb.tile([128, FT, KC], bf16)
                w_v = xwb.tile([128, FT, DIM], bf16)
                w_k = xwb.tile([128, FT, DIM], bf16)
                # sync-queue order = need order: (xT,wq) chunk pairs so Q can
                # start on the first 128-row chunk, then (xkv,wv) pairs for
                # the chunk-paced V passes, then wk, then the rest.
                sb_xTb = xwa.tile([128, FT, BLK], bf16)
                w_q = xwa.tile([128, FT, DIM], bf16)
                for kt in range(FT):
                    nc.sync.dma_start(
                        out=sb_xTb[:, kt, :],
                        in_=d_xTb[kt * 128 : (kt + 1) * 128, :],
                    )
                    nc.sync.dma_start(
                        out=w_q[:, kt, :], in_=d_wq[kt * 128 : (kt + 1) * 128, :]
                    )
                # the xkv/wv stream rides the gpsimd sequencer: the DMA
                # engines reach 340+GB/s once fed, but each sequencer only
                # issues a descriptor every ~0.7us — two rings issue in
                # parallel, so V's inputs land while sync still feeds Q's
                for kt in range(FT):
                    nc.gpsimd.dma_start(
                        out=sb_xkvT[:, kt, :],
                        in_=d_xkvT[kt * 128 : (kt + 1) * 128, :],
                    )
                    nc.gpsimd.dma_start(
                        out=w_v[:, kt, :], in_=d_wv[kt * 128 : (kt + 1) * 128, :]
                    )
                for kt in range(FT):
                    nc.sync.dma_start(
                        out=w_k[:, kt, :], in_=d_wk[kt * 128 : (kt + 1) * 128, :]
                    )
                nc.sync.dma_start(
                    out=sb_xblk, in_=d_xb[:].rearrange("(t p) d -> p t d", p=128)
                )
                for kt in range(FT):
                    nc.sync.dma_start(
                        out=w_o[:, kt, :], in_=d_wo[kt * 128 : (kt + 1) * 128, :]
                    )
                for kt in range(FT):
                    nc.sync.dma_start(
                        out=w1_t[kt], in_=d_w1[kt * 128 : (kt + 1) * 128, :]
                    )
                for kt in range(W2A):
                    nc.sync.dma_start(
                        out=w2_t[kt], in_=d_w2[kt * 128 : (kt + 1) * 128, :]
                    )

                # ones column of V: 12 cheap strided copies from the dense
                # tok-major validity tile (a direct strided DMA generates
                # 2-byte-packet descriptor spam and a ~26us drain)
                for h in range(HEADS):
                    nc.vector.tensor_copy(
                        sb_V[:, :, h, DK : DK + 1],
                        sb_onescT[:].rearrange("p (t o) -> p t o", o=1),
                    )

                # Q^T feat-major, kt-outer so compute starts on chunk 0
                with tc.tile_pool(name="ps_q", bufs=1, space="PSUM") as ps_q:
                    psq = [
                        ps_q.tile([128, BLK], f32, tag=f"q{ft}", name=f"psq{ft}")
                        for ft in range(FT)
                    ]
                    for kt in range(FT):
                        for ft in range(FT):
                            nc.tensor.matmul(
                                psq[ft],
                                w_q[:, kt, ft * 128 : (ft + 1) * 128],
                                sb_xTb[:, kt, :],
                                start=(kt == 0),
                                stop=(kt == FT - 1),
                            )
                    for ft in range(FT):
                        nc.scalar.activation(
                            sb_Q[:, ft, :], psq[ft], AF.Identity,
                            bias=sb_bqT[:, ft : ft + 1],
                        )

                # V tok-major over compacted keys, kt-outer in chunk-paced
                # passes of <=8 concurrent PSUM groups
                with tc.tile_pool(name="ps_v", bufs=1, space="PSUM") as ps_v:
                    for vpass in VPASS:
                        psv = [
                            ps_v.tile([128, 384], f32, tag=f"v{i}", name=f"psv{i}")
                            for i in range(len(vpass))
                        ]
                        for kt in range(FT):
                            for i, (nh, tt) in enumerate(vpass):
                                nc.tensor.matmul(
                                    psv[i],
                                    sb_xkvT[:, kt, tt * 128 : (tt + 1) * 128],
                                    w_v[:, kt, nh * 384 : (nh + 1) * 384],
                                    start=(kt == 0),
                                    stop=(not general and kt == FT - 1),
                                )
                        for i, (nh, tt) in enumerate(vpass):
                            if general:
                                nc.tensor.matmul(
                                    psv[i],
                                    sb_onesc[0:1, tt * 128 : (tt + 1) * 128],
                                    sb_bvrow[0:1, nh * 384 : (nh + 1) * 384],
                                    start=False,
                                    stop=True,
                                )
                            nc.vector.tensor_copy(
                                sb_V[:, tt, nh * 6 : (nh + 1) * 6, 0:DK],
                                psv[i][:].rearrange("p (h d) -> p h d", d=DK),
                            )

                with (
                    tc.tile_pool(name="ps1", bufs=2, space="PSUM") as ps1,
                    tc.tile_pool(name="ps_sc", bufs=4, space="PSUM") as ps_sc,
                    tc.tile_pool(name="ps_z", bufs=1, space="PSUM") as ps_z,
                ):
                    def k_proj(ft):
                        for c0, c1 in KCH:
                            ps = ps1.tile([128, c1 - c0], f32, tag="p", name="ps_k")
                            for kt in range(FT):
                                nc.tensor.matmul(
                                    ps,
                                    w_k[:, kt, ft * 128 : (ft + 1) * 128],
                                    sb_xkvT[:, kt, c0:c1],
                                    start=(kt == 0),
                                    stop=(not general and kt == FT - 1),
                                )
                            if general:
                                nc.tensor.matmul(
                                    ps,
                                    sb_bkrow[0:1, ft * 128 : (ft + 1) * 128],
                                    sb_onesc[0:1, c0:c1],
                                    start=False,
                                    stop=True,
                                )
                            nc.vector.tensor_copy(sb_K[:, ft, c0:c1], ps)

                    def scores(hp):
                        ets = ([], [])
                        for kt2 in range(STC):
                            # one [128,512] PSUM group per half: 4-deep
                            # rotation so the exp() pipeline never stalls
                            # the PE in the final (uninterleaved) rounds
                            for half in (0, 1):
                                ho = half * 64
                                ps = ps_sc.tile([128, BLK], f32, tag="sc",
                                                name="ps_s")
                                nc.tensor.matmul(
                                    ps,
                                    sb_K[ho : ho + 64, hp,
                                         kt2 * 128 : (kt2 + 1) * 128],
                                    sb_Q[ho : ho + 64, hp, :],
                                    start=True,
                                    stop=True,
                                )
                                et = expp.tile([128, BLK], bf16, tag="exp",
                                               name="et")
                                nc.scalar.activation(et, ps, AF.Exp,
                                                     scale=ISCALE)
                                ets[half].append(et[:])
                        return ets

                    def attn_v(hp, ets):
                        # per-half: matmuls then immediately the softmax tail,
                        # so half 0's tail overlaps half 1's matmuls
                        for half in (0, 1):
                            h = 2 * hp + half
                            ho = half * 64
                            zp = ps_z.tile([DK + 1, BLK], f32, tag=f"z{half}",
                                           name="ps_z")
                            # seed with the masked-keys correction row
                            nc.tensor.matmul(
                                zp,
                                sb_crow[0:1, h * (DK + 1) : (h + 1) * (DK + 1)],
                                ones512b[:],
                                start=True,
                                stop=False,
                            )
                            for kt2 in range(STC):
                                nc.tensor.matmul(
                                    zp,
                                    sb_V[:, kt2, h, :],
                                    ets[half][kt2],
                                    start=False,
                                    stop=(kt2 == STC - 1),
                                )
                            # denominators are huge sums (>= 1): the ~18-bit fast
                            # reciprocal is ~5x cheaper and plenty exact.
                            # (bitwise-trick op: input must be in SBUF, not PSUM)
                            zrow = attsm.tile([1, BLK], f32, tag="zrow", name="zrow")
                            nc.vector.tensor_copy(zrow, zp[DK : DK + 1, :])
                            rsum = attsm.tile([1, BLK], f32, tag="rsum", name="rsum")
                            nc.vector.reciprocal_approx_fast(rsum, zrow)
                            # fan the [1,512] reciprocal out to 64 partitions on
                            # the (otherwise idle) GpSimd engine — no PE matmul,
                            # no PSUM bank, no DVE copy
                            rb = attsm.tile([64, BLK], f32, tag="rbs", name="rb")
                            nc.gpsimd.partition_broadcast(rb[:], rsum[:])
                            nc.vector.tensor_mul(
                                sb_zT[hp][ho : ho + 64, :], zp[0:DK, :], rb
                            )

                    k_proj(0)
                    ets_prev = scores(0)
                    for hp in range(1, HEADS // 2):
                        k_proj(hp)
                        attn_v(hp - 1, ets_prev)
                        ets_prev = scores(hp)
                    attn_v(HEADS // 2 - 1, ets_prev)

            kqv_cm.__exit__(None, None, None)

            # broadcast-row constants + the rest of w2, loaded into the space
            # the attention residents just freed
            const2_cm = tc.tile_pool(name="const2", bufs=1)
            const2 = const2_cm.__enter__()
            bo_bc = const2.tile([128, DIM], bf16)
            nc.gpsimd.dma_start(out=bo_bc, in_=bcast_ap(d_bo))
            b2_bc = const2.tile([128, DIM], bf16)
            nc.gpsimd.dma_start(out=b2_bc, in_=bcast_ap(d_b2))
            g1_bc = const2.tile([128, DIM], bf16)
            nc.gpsimd.dma_start(out=g1_bc, in_=bcast_ap(d_g1))
            bb1_bc = const2.tile([128, DIM], bf16)
            nc.gpsimd.dma_start(out=bb1_bc, in_=bcast_ap(d_bb1))
            g2_bc = const2.tile([128, DIM], bf16)
            nc.gpsimd.dma_start(out=g2_bc, in_=bcast_ap(d_g2))
            bb2_bc = const2.tile([128, DIM], bf16)
            nc.gpsimd.dma_start(out=bb2_bc, in_=bcast_ap(d_bb2))
            ident = const2.tile([128, 128], bf16)
            make_identity(nc, ident[:])

            w2b_cm = tc.tile_pool(name="w2b", bufs=1)
            w2b = w2b_cm.__enter__()
            for kt in range(W2A, HT):
                w2_kt = w2b.tile([128, DIM], bf16, tag=f"w2_{kt}", name=f"w2_{kt}")
                nc.sync.dma_start(out=w2_kt, in_=d_w2[kt * 128 : (kt + 1) * 128, :])
                w2_t.append(w2_kt)

            # ===== Phases 3-5 merged: per-token-tile pipeline =====
            # s (the row-sum of x) comes from accum_out on the producer's
            # bias-add, saving a [128,768] reduce on the LN critical path
            def ln_stats(x_ap, s, pool):
                junk = pool.tile([128, DIM], bf16, tag="ln_j")
                ssq = pool.tile([128, 1], f32, tag="ln_q")
                # x^2 row-sums on the ACT engine (accum_out) — keeps the
                # [128,768] pass off the DVE queue, which is the block
                # bottleneck
                nc.scalar.activation(junk, x_ap, AF.Square, accum_out=ssq)
                negmean = pool.tile([128, 1], f32, tag="ln_m")
                nc.vector.tensor_scalar_mul(negmean, s, -1.0 / DIM)
                m2 = pool.tile([128, 1], f32, tag="ln_m2")
                nc.vector.tensor_mul(m2, negmean, negmean)
                return ssq, negmean, m2

            def ln_apply(out_ap, x_ap, g_bc_t, resid_ap, pool, ssq, negmean,
                         m2):
                # var = E[x^2] - mean^2
                var = pool.tile([128, 1], f32, tag="ln_v")
                nc.vector.scalar_tensor_tensor(
                    out=var, in0=ssq, scalar=1.0 / DIM, in1=m2,
                    op0=ALU.mult, op1=ALU.subtract,
                )
                sd = pool.tile([128, 1], f32, tag="ln_sd")
                nc.scalar.activation(sd, var, AF.Sqrt, bias=eps_t[:])
                rstd = pool.tile([128, 1], f32, tag="ln_r")
                nc.vector.reciprocal(rstd, sd)
                nmr = pool.tile([128, 1], f32, tag="ln_nm")
                nc.vector.tensor_mul(nmr, negmean, rstd)
                if not general:
                    # g==1, b==0: normalized = (x*rstd)+nmr as ONE DVE
                    # tensor_scalar with two per-partition scalars — keeps
                    # the chain on one engine (each cross-engine sem hop
                    # measures 0.3-0.8us)
                    t1 = pool.tile([128, DIM], f32, tag="ln_t")
                    nc.vector.tensor_scalar(
                        out=t1, in0=x_ap, scalar1=rstd[:], scalar2=nmr[:],
                        op0=ALU.mult, op1=ALU.add,
                    )
                    nc.vector.tensor_add(out_ap, t1, resid_ap)
                    return
                # x*rstd on ACT; then (x*rstd - mu*rstd) * g fused on DVE
                nrm = pool.tile([128, DIM], f32, tag="ln_t")
                nc.scalar.mul(nrm, x_ap, rstd[:])
                tg = pool.tile([128, DIM], f32, tag="ln_tg")
                nc.vector.scalar_tensor_tensor(
                    out=tg, in0=nrm, scalar=nmr[:], in1=g_bc_t,
                    op0=ALU.add, op1=ALU.mult,
                )
                nc.vector.tensor_add(out_ap, tg, resid_ap)

            def layer_norm_to(out_ap, x_ap, g_bc_t, resid_ap, pool, s):
                ssq, negmean, m2 = ln_stats(x_ap, s, pool)
                ln_apply(out_ap, x_ap, g_bc_t, resid_ap, pool, ssq, negmean,
                         m2)

            # all four O-projs + LN1s first: LN1 latency hides under them.
            # kt 0..4 accumulate two token-tiles ahead; the kt=5 finisher
            # (which depends on the last head-pair's softmax tail) trails,
            # so the PE never waits on that tail.
            with (
                tc.tile_pool(name="ln1p", bufs=4) as ln1p,
                tc.tile_pool(name="ps_o", bufs=2, space="PSUM") as ps_o,
            ):
                ops = {}

                def o_start(tt):
                    prs = []
                    for nh in range(2):
                        ps = ps_o.tile([128, 384], f32, tag=f"o{nh}",
                                       name=f"ps_o{nh}")
                        for kt in range(FT - 1):
                            nc.tensor.matmul(
                                ps,
                                sb_zT[kt][:, tt * 128 : (tt + 1) * 128],
                                w_o[:, kt, nh * 384 : (nh + 1) * 384],
                                start=(kt == 0),
                                stop=False,
                            )
                        prs.append(ps)
                    ops[tt] = prs

                pres = {}

                def o_finish(tt):
                    # kt=5 finishers + bias-add STTs only; the LN chains are
                    # issued after ALL of these so their ACT<->DVE ping-pong
                    # latency never blocks the later STTs at the DVE queue
                    # head (which would stall the O PSUM rotation)
                    prs = ops.pop(tt)
                    l1pre = ln1p.tile([128, DIM], f32, tag="pre")
                    accs = []
                    for nh in range(2):
                        nc.tensor.matmul(
                            prs[nh],
                            sb_zT[FT - 1][:, tt * 128 : (tt + 1) * 128],
                            w_o[:, FT - 1, nh * 384 : (nh + 1) * 384],
                            start=False,
                            stop=True,
                        )
                        acc = ln1p.tile([128, 1], f32, tag=f"acc{nh}",
                                        name=f"acc{nh}")
                        nc.vector.scalar_tensor_tensor(
                            out=l1pre[:, nh * 384 : (nh + 1) * 384],
                            in0=prs[nh],
                            scalar=1.0,
                            in1=bo_bc[:, nh * 384 : (nh + 1) * 384],
                            op0=ALU.mult,
                            op1=ALU.add,
                            accum_out=acc,
                        )
                        accs.append(acc)
                    s = ln1p.tile([128, 1], f32, tag="ln_s")
                    nc.vector.tensor_add(s, accs[0], accs[1])
                    pres[tt] = (l1pre, ln_stats(l1pre[:], s, ln1p))

                o_start(0)
                o_start(1)
                o_finish(0)
                o_start(2)
                o_finish(1)
                o_start(3)
                o_finish(2)
                o_finish(3)
                for tt in range(TT):
                    l1pre, stats = pres.pop(tt)
                    if general:
                        xb1 = ln1p.tile([128, DIM], f32, tag="resid")
                        nc.vector.tensor_add(xb1, sb_xblk[:, tt, :], bb1_bc)
                        resid = xb1[:]
                    else:
                        resid = sb_xblk[:, tt, :]
                    ln_apply(sb_l1[:, tt, :], l1pre[:], g1_bc, resid, ln1p,
                             *stats)

            # per token tile: transpose -> FFN1 -> FFN2 -> LN2 -> store
            with (
                tc.tile_pool(name="ln2p", bufs=4) as ln2p,
                tc.tile_pool(name="l1tp", bufs=1) as l1tp,
                tc.tile_pool(name="hTp", bufs=1) as hTp,
                tc.tile_pool(name="outp", bufs=2) as outp,
                tc.tile_pool(name="ps_t", bufs=3, space="PSUM") as ps_t,
                tc.tile_pool(name="ps_f1", bufs=3, space="PSUM") as ps_f1,
                tc.tile_pool(name="ps_f2", bufs=2, space="PSUM") as ps_f2,
            ):
                out_r = d_out[:].rearrange("(t p) d -> p t d", p=128)
                # full-width l1^T, transposed per token tile as its LN1
                # lands; FFN1 then runs 24 N=512 groups (the ~200ns
                # per-group boundary tax made 96 N=128 groups cost ~14us
                # more)
                l1T = l1tp.tile([128, FT, BLK], bf16, tag="l1T", name="l1T")
                for tt in range(TT):
                    for ft in range(FT):
                        pst = ps_t.tile([128, 128], bf16, tag="tp")
                        nc.tensor.transpose(
                            pst, sb_l1[:, tt, ft * 128 : (ft + 1) * 128], ident[:]
                        )
                        ceng = nc.vector if ft % 2 else nc.scalar
                        if ceng is nc.vector:
                            nc.vector.tensor_copy(
                                l1T[:, ft, tt * 128 : (tt + 1) * 128], pst
                            )
                        else:
                            nc.scalar.copy(
                                l1T[:, ft, tt * 128 : (tt + 1) * 128], pst
                            )
                hT = hTp.tile([128, HT, BLK], bf16, tag="hT", name="hT")
                for ht2 in range(HT):
                    ps = ps_f1.tile([128, BLK], f32, tag="f1")
                    for kt in range(FT):
                        nc.tensor.matmul(
                            ps,
                            w1_t[kt][:, ht2 * 128 : (ht2 + 1) * 128],
                            l1T[:, kt, :],
                            start=(kt == 0),
                            stop=(kt == FT - 1),
                        )
                    # relu(x + b1) on ACT (per-partition bias)
                    nc.scalar.activation(
                        hT[:, ht2, :], ps, AF.Relu,
                        bias=sb_b1[:, ht2 : ht2 + 1],
                    )
                for tt in range(TT):
                    f2pre = ln2p.tile([128, DIM], f32, tag="pre")
                    accs = []
                    for nh in range(2):
                        ps = ps_f2.tile([128, 384], f32, tag="f2")
                        for kt in range(HT):
                            nc.tensor.matmul(
                                ps,
                                hT[:, kt, tt * 128 : (tt + 1) * 128],
                                w2_t[kt][:, nh * 384 : (nh + 1) * 384],
                                start=(kt == 0),
                                stop=(kt == HT - 1),
                            )
                        acc = ln2p.tile([128, 1], f32, tag=f"acc{nh}",
                                        name=f"acc{nh}")
                        nc.vector.scalar_tensor_tensor(
                            out=f2pre[:, nh * 384 : (nh + 1) * 384],
                            in0=ps,
                            scalar=1.0,
                            in1=b2_bc[:, nh * 384 : (nh + 1) * 384],
                            op0=ALU.mult,
                            op1=ALU.add,
                            accum_out=acc,
                        )
                        accs.append(acc)
                    s = ln2p.tile([128, 1], f32, tag="ln_s")
                    nc.vector.tensor_add(s, accs[0], accs[1])
                    if general:
                        l1b = ln2p.tile([128, DIM], f32, tag="resid")
                        nc.vector.tensor_add(l1b, sb_l1[:, tt, :], bb2_bc)
                        resid = l1b[:]
                    else:
                        resid = sb_l1[:, tt, :]
                    o_sb = outp.tile([128, DIM], f32, tag="osb")
                    layer_norm_to(o_sb[:], f2pre[:], g2_bc, resid, ln2p, s)
                    nc.sync.dma_start(out=out_r[:, tt, :], in_=o_sb)

            w2b_cm.__exit__(None, None, None)
            const2_cm.__exit__(None, None, None)
            zT_cm.__exit__(None, None, None)
            wpre_cm.__exit__(None, None, None)

    return nc


def _get_nc(general: bool = False, finalized=True):
    key = f"nc{int(general)}"
    if key not in _CACHE:
        _CACHE[key] = _build_program(general)
    nc = _CACHE[key]
    if finalized and not nc.is_finalized():
        nc.finalize()
    return nc


def make_in_maps(inputs: dict) -> list:
    x = np.asarray(inputs["x_n"], np.float32).reshape(B, S, DIM)
    mask = np.asarray(inputs["mask"]).reshape(B, S)
    w = {
        k: np.ascontiguousarray(np.asarray(inputs[k], np.float32).astype(BF16))
        for k in ("wq", "wk", "wv", "wo", "w1", "w2")
    }
    vecs = {
        "bo": inputs["bo"], "b1": inputs["b1"], "b2": inputs["b2"],
        "g1": inputs["ln1_g"], "bb1": inputs["ln1_b"],
        "g2": inputs["ln2_g"], "bb2": inputs["ln2_b"],
        "bqT": inputs["bq"],
    }
    vecs = {k: np.ascontiguousarray(np.asarray(v, np.float32)) for k, v in vecs.items()}
    brows = {
        "bkrow": np.asarray(inputs["bk"], np.float32).astype(BF16),
        "bvrow": np.asarray(inputs["bv"], np.float32).astype(BF16),
    }

    # per-batch compaction + masked-keys correction
    per_batch = []
    for b in range(B):
        mb = mask[b] != 0
        idx = np.nonzero(mb)[0]
        n_u = len(idx)
        if n_u > KC:
            raise RuntimeError(
                f"unmasked key count {n_u} exceeds compiled capacity {KC}"
            )
        xkv = np.zeros((KC, DIM), np.float32)
        xkv[:n_u] = x[b][idx]
        xkvT = np.ascontiguousarray(xkv.T.astype(BF16))
        onesc = np.zeros(KC, np.float32)
        onesc[:n_u] = 1.0
        msum = x[b][~mb].astype(np.float64).sum(axis=0)
        mcount = float((~mb).sum())
        wv64 = np.asarray(inputs["wv"], np.float64)
        bv64 = np.asarray(inputs["bv"], np.float64)
        cvec = (msum @ wv64 + mcount * bv64).astype(np.float32)  # [DIM]
        crow = np.zeros(HEADS * (DK + 1), np.float32)
        ch = cvec.reshape(HEADS, DK)
        for h in range(HEADS):
            crow[h * (DK + 1) : h * (DK + 1) + DK] = ch[h]
            crow[h * (DK + 1) + DK] = mcount
        per_batch.append(
            {"xkvT": xkvT, "onesc": onesc.astype(BF16), "crow": crow.astype(BF16)}
        )

    in_maps = []
    for c in range(N_CORES):
        b, blk = c // NBLK, c % NBLK
        xb = x[b]
        xblk = np.ascontiguousarray(xb[blk * BLK : (blk + 1) * BLK])
        xTb = np.ascontiguousarray(xblk.T.astype(BF16))
        m = {"xTb": xTb, "xb": xblk}
        m.update(per_batch[b])
        m.update(w)
        m.update(vecs)
        m.update(brows)
        in_maps.append(m)
    return in_maps


def assemble(per_core_out: list) -> np.ndarray:
    blocks = [np.asarray(o, np.float32) for o in per_core_out]
    full = np.concatenate(blocks, axis=0).reshape(B, S, DIM)
    return full


def kernel(**inputs) -> np.ndarray:
    from concourse.bass_utils import run_bass_kernel_spmd

    general = bool(
        np.any(np.asarray(inputs["bk"]))
        or np.any(np.asarray(inputs["bv"]))
        or np.any(np.asarray(inputs["ln1_b"]))
        or np.any(np.asarray(inputs["ln2_b"]))
        or np.any(np.asarray(inputs["ln1_g"]) != 1)
        or np.any(np.asarray(inputs["ln2_g"]) != 1)
    )
    nc = _get_nc(general)
    in_maps = make_in_maps(inputs)
    res = run_bass_kernel_spmd(nc, in_maps, list(range(N_CORES)))
    return assemble([r["out"] for r in res.results])


# revision 25
# speedup vs baseline: 1.2776x; 1.2776x over previous
"""Trainium2 Bass kernel for a dense transformer encoder layer.

Model (faithful to the oracle):
  q,k,v = x@wq+bq, x@wk+bk, x@wv+bv          (12 heads, dk=64, DIM=768)
  scores = q@k^T / sqrt(768)  (note: sqrt(dim_model), not sqrt(dk))
  scores[mask==0] = 1e-11  (NOT -inf; masked keys still contribute ~1/Z)
  attn = softmax(scores); z = attn@v; o = z@wo+bo
  l1 = x + LN(o);  ffn = relu(l1@w1+b1)@w2+b2;  out = l1 + LN(ffn)

Sharding: 4096 tokens (B=2,S=2048) split 8 ways -> 512 tokens/core.
Cores 0-3 own batch 0, cores 4-7 batch 1. No collectives: each core
computes K/V itself — but only for the UNMASKED keys of its batch.

Mask compaction: masked keys (~half) all get score 1e-11, i.e. exp==1
(fp32), for every query/head. Their attention contribution is a
query-independent constant: C_h = sum_masked v_k (numerator) and
m = #masked (denominator). The host compacts unmasked tokens into a
padded [KC] buffer, computes the tiny C correction in numpy, and the
device runs attention only over the compacted keys, seeding the attn@v
PSUM accumulation with C via a K=1 bf16 matmul. Padded key slots have
K=V=0 exactly (biases are added inside the matmuls via a 0/1 validity
row), so exp(score)=1 * V=0 contributes nothing; the validity row also
zeroes their denominator entry.

Everything stays bf16: fp8 anywhere in the attention path (q/k/v
projections, stored V, exp weights) measurably costs ~1.3e-2 relative
error each — the attention output is a near-uniform average over ~2k
keys, so its *signal* is small and per-element quantization noise does
NOT average away relative to it.

Perf changes vs the 256.9us v2 baseline (now ~219us):
 - Q projection is kt-outer over 6 PSUM banks and the x^T/wq DMAs are
   interleaved per 128-row chunk, so the first matmul starts ~10us in
   instead of ~23us; bq is added on the ACT PSUM->SBUF copy
   (per-partition bias) instead of a PE matmul.
 - V projection runs kt-outer in 3 passes of <=8 PSUM banks so it can
   chew each (xkv,wv) chunk as it lands instead of waiting for all six.
 - The C-seed matmuls are bf16 (were fp32 = 4 cycles/row on the PE).
 - z^T lives in one tile per head-pair, so the O-projection's kt<5
   matmuls start as soon as the last head-pair's attn@v PE work ends
   instead of serializing behind its softmax tail; the kt=5 finishers
   trail two token-tiles behind the kt0-4 accumulations.
 - attn@v emits each half's softmax tail (reciprocal + gpsimd
   partition-broadcast) immediately, overlapping the other half.
 - Phases 3-5 are merged into one per-token-tile pipeline
   (O-proj -> LN1 -> transpose -> FFN1 -> FFN2 -> LN2 -> store) with
   relu+bias on the ACT engine (per-partition b1); LN row-sums come
   from accum_out on the producers' bias-add STTs; transposes for
   tile tt+1 are issued before FFN2(tt).
 - l1 is kept in bf16 (residual quantization ~0.2%), making the
   transposes 1 cycle/row instead of 2 (fp32).
 - All of w2 is prefetched (12 tiles behind w1 on the sync queue, 12
   right after attention) so FFN2 of token tile 0 never waits.
 - When bk/bv are all-zero (host-checked), a program variant skips the
   K/V bias matmuls; the general variant compiles on demand otherwise.

Further (this file): FFN1 runs as 24 N=512 groups off a full-width
l1^T (the ~200ns per-matmul-GROUP boundary tax made 96 N=128 groups
~14us slower); LN row-sums via accum_out, x^2 sums via ACT
Square+accum_out; when biases AND LayerNorm affines are trivial
(host-checked, the general variant compiles on demand), the K/V bias
matmuls and affine/residual-bias ops are skipped and the normalize
collapses to one ACT op (per-partition scale+bias) + one add; the LN
pools run bufs=4 and the chains are issued AFTER all O-proj
STTs so their ACT<->DVE ping-pong never blocks the DVE queue head.

Measured-but-reverted (regressions): fp8 anywhere in attention
(error), bootstrapping DMAs on the gpsimd queue (that ring starts
LATER than sync for bulk), bf16 partition-broadcasts (slower), FFN1
at N=256 token-tile pairs (deepens the LN1->transpose barrier),
ps_o bufs=4 (no effect on the DVE-serialized O epilogue).
"""

import math
import os
import sys

import numpy as np

for _p in ("/opt/trn_rl_repo", os.path.expanduser("~/.axon_site/_ro/trn_rl_repo")):
    if os.path.isdir(_p) and _p not in sys.path:
        sys.path.insert(0, _p)

import ml_dtypes  # noqa: E402

BF16 = ml_dtypes.bfloat16

DIM = 768
HEADS = 12
DK = 64
HID = 4 * DIM  # 3072
B, S = 2, 2048
N_CORES = 8
BLK = 512            # tokens per core
NBLK = S // BLK      # 4 blocks per batch
KC = 1152            # compacted-key capacity (9 tiles; ~5.7 sigma above E[n_u])
EPS = 1e-5
ISCALE = 1.0 / math.sqrt(DIM)

FT = DIM // 128   # 6 feature tiles
TT = BLK // 128   # 4 token tiles per core block
STC = KC // 128   # 9 compacted key tiles
HT = HID // 128   # 24 hidden tiles
W2A = 12          # w2 tiles prefetched during attention

_CACHE: dict = {}


def _build_program(general: bool):
    import concourse.bass as bass
    import concourse.mybir as mybir
    import concourse.tile as tile
    from concourse import bacc
    from concourse.masks import make_identity

    f32 = mybir.dt.float32
    bf16 = mybir.dt.bfloat16
    AF = mybir.ActivationFunctionType
    ALU = mybir.AluOpType
    AX = mybir.AxisListType

    nc = bacc.Bacc()

    # ---- per-core DRAM I/O ----
    d_xTb = nc.dram_tensor("xTb", [DIM, BLK], bf16, kind="ExternalInput")
    d_xb = nc.dram_tensor("xb", [BLK, DIM], f32, kind="ExternalInput")
    d_xkvT = nc.dram_tensor("xkvT", [DIM, KC], bf16, kind="ExternalInput")
    d_onesc = nc.dram_tensor("onesc", [KC], bf16, kind="ExternalInput")
    d_wq = nc.dram_tensor("wq", [DIM, DIM], bf16, kind="ExternalInput")
    d_wk = nc.dram_tensor("wk", [DIM, DIM], bf16, kind="ExternalInput")
    d_wv = nc.dram_tensor("wv", [DIM, DIM], bf16, kind="ExternalInput")
    d_wo = nc.dram_tensor("wo", [DIM, DIM], bf16, kind="ExternalInput")
    d_w1 = nc.dram_tensor("w1", [DIM, HID], bf16, kind="ExternalInput")
    d_w2 = nc.dram_tensor("w2", [HID, DIM], bf16, kind="ExternalInput")
    d_bkrow = nc.dram_tensor("bkrow", [DIM], bf16, kind="ExternalInput")
    d_bvrow = nc.dram_tensor("bvrow", [DIM], bf16, kind="ExternalInput")
    d_bqT = nc.dram_tensor("bqT", [DIM], f32, kind="ExternalInput")
    d_crow = nc.dram_tensor("crow", [HEADS * (DK + 1)], bf16, kind="ExternalInput")
    d_bo = nc.dram_tensor("bo", [DIM], f32, kind="ExternalInput")
    d_b1 = nc.dram_tensor("b1", [HID], f32, kind="ExternalInput")
    d_b2 = nc.dram_tensor("b2", [DIM], f32, kind="ExternalInput")
    d_g1 = nc.dram_tensor("g1", [DIM], f32, kind="ExternalInput")
    d_bb1 = nc.dram_tensor("bb1", [DIM], f32, kind="ExternalInput")
    d_g2 = nc.dram_tensor("g2", [DIM], f32, kind="ExternalInput")
    d_bb2 = nc.dram_tensor("bb2", [DIM], f32, kind="ExternalInput")
    d_out = nc.dram_tensor("out", [BLK, DIM], f32, kind="ExternalOutput")

    KCH = [(0, 512), (512, 1024), (1024, KC)]  # K-proj N chunks
    # V kt-outer passes: (nh, tt) groups, <=8 PSUM banks at a time
    VGRP = [(nh, tt) for nh in range(2) for tt in range(STC)]
    VPASS = [VGRP[0:8], VGRP[8:16], VGRP[16:18]]

    def bcast_ap(handle, n=128):
        ap = handle[:]
        return bass.AP(tensor=ap.tensor, offset=ap.offset, ap=[[0, n]] + list(ap.ap))

    with tile.TileContext(nc) as tc:
        with (
            tc.tile_pool(name="const", bufs=1) as const,
            tc.tile_pool(name="bigres", bufs=1) as big,
        ):
            # ---------- small constants (gpsimd queue) ----------
            sb_b1 = const.tile([128, HT], f32)
            nc.gpsimd.dma_start(
                out=sb_b1, in_=d_b1[:].rearrange("(t p) -> p t", p=128)
            )
            sb_bqT = const.tile([128, FT], f32)
            nc.gpsimd.dma_start(
                out=sb_bqT, in_=d_bqT[:].rearrange("(t p) -> p t", p=128)
            )
            sb_bkrow = const.tile([1, DIM], bf16)
            nc.gpsimd.dma_start(out=sb_bkrow, in_=bcast_ap(d_bkrow, 1))
            sb_bvrow = const.tile([1, DIM], bf16)
            nc.gpsimd.dma_start(out=sb_bvrow, in_=bcast_ap(d_bvrow, 1))
            sb_crow = const.tile([1, HEADS * (DK + 1)], bf16)
            nc.gpsimd.dma_start(out=sb_crow, in_=bcast_ap(d_crow, 1))
            sb_onesc = const.tile([1, KC], bf16)
            nc.gpsimd.dma_start(out=sb_onesc, in_=bcast_ap(d_onesc, 1))
            sb_onescT = const.tile([128, STC], bf16)
            nc.gpsimd.dma_start(
                out=sb_onescT, in_=d_onesc[:].rearrange("(t p) -> p t", p=128)
            )
            ones512b = const.tile([1, BLK], bf16)
            nc.vector.memset(ones512b, 1.0)
            eps_t = const.tile([128, 1], f32)
            nc.vector.memset(eps_t, EPS)

            # ---------- persistent activations ----------
            sb_xblk = big.tile([128, TT, DIM], f32)  # residual x
            sb_l1 = big.tile([128, TT, DIM], bf16)

            # FFN/O weights: loaded behind the attention inputs on the sync
            # queue, consumed post-attention
            wpre_cm = tc.tile_pool(name="wpre", bufs=1)
            wpre = wpre_cm.__enter__()
            w_o = wpre.tile([128, FT, DIM], bf16)
            w1_t = []
            for kt in range(FT):
                w1_kt = wpre.tile([128, HID], bf16, tag=f"w1_{kt}", name=f"w1_{kt}")
                w1_t.append(w1_kt)
            w2_t = []
            for kt in range(W2A):
                w2_kt = wpre.tile([128, DIM], bf16, tag=f"w2_{kt}", name=f"w2_{kt}")
                w2_t.append(w2_kt)

            # z^T outlives the K/Q/V residents (consumed by the O-proj).
            # One tile per head-pair so O-proj reads of early head-pairs
            # don't serialize behind the last head-pair's softmax tail.
            zT_cm = tc.tile_pool(name="zTp", bufs=1)
            zTp = zT_cm.__enter__()
            sb_zT = [
                zTp.tile([128, BLK], bf16, tag=f"zT{hp}", name=f"zT{hp}")
                for hp in range(HEADS // 2)
            ]

            kqv_cm = tc.tile_pool(name="kqvp", bufs=1)
            kqvp = kqv_cm.__enter__()
            sb_K = kqvp.tile([128, FT, KC], bf16)  # K^T, feat-major
            sb_Q = kqvp.tile([128, FT, BLK], bf16)  # Q^T, feat-major
            sb_V = kqvp.tile([128, STC, HEADS, DK + 1], bf16)  # V + ones col

            # ===== Phase 1+2: QKV projections fused with attention =====
            with (
                tc.tile_pool(name="xw_b", bufs=1) as xwb,
                tc.tile_pool(name="xw_a", bufs=1) as xwa,
                tc.tile_pool(name="expp", bufs=10) as expp,
                tc.tile_pool(name="attsm", bufs=1) as attsm,
            ):
                sb_xkvT = xwb.tile([128, FT, KC], bf16)
                w_v = xwb.tile([128, FT, DIM], bf16)
                w_k = xwb.tile([128, FT, DIM], bf16)
                # sync-queue order = need order: (xT,wq) chunk pairs so Q can
                # start on the first 128-row chunk, then (xkv,wv) pairs for
                # the chunk-paced V passes, then wk, then the rest.
                sb_xTb = xwa.tile([128, FT, BLK], bf16)
                w_q = xwa.tile([128, FT, DIM], bf16)
                for kt in range(FT):
                    nc.sync.dma_start(
                        out=sb_xTb[:, kt, :],
                        in_=d_xTb[kt * 128 : (kt + 1) * 128, :],
                    )
                    nc.sync.dma_start(
                        out=w_q[:, kt, :], in_=d_wq[kt * 128 : (kt + 1) * 128, :]
                    )
                # the xkv/wv stream rides the gpsimd sequencer: the DMA
                # engines reach 340+GB/s once fed, but each sequencer only
                # issues a descriptor every ~0.7us — two rings issue in
                # parallel, so V's inputs land while sync still feeds Q's
                for kt in range(FT):
                    nc.gpsimd.dma_start(
                        out=sb_xkvT[:, kt, :],
                        in_=d_xkvT[kt * 128 : (kt + 1) * 128, :],
                    )
                    nc.gpsimd.dma_start(
                        out=w_v[:, kt, :], in_=d_wv[kt * 128 : (kt + 1) * 128, :]
                    )
                for kt in range(FT):
                    nc.sync.dma_start(
                        out=w_k[:, kt, :], in_=d_wk[kt * 128 : (kt + 1) * 128, :]
                    )
                nc.sync.dma_start(
                    out=sb_xblk, in_=d_xb[:].rearrange("(t p) d -> p t d", p=128)
                )
                for kt in range(FT):
                    nc.sync.dma_start(
                        out=w_o[:, kt, :], in_=d_wo[kt * 128 : (kt + 1) * 128, :]
                    )
                for kt in range(FT):
                    nc.sync.dma_start(
                        out=w1_t[kt], in_=d_w1[kt * 128 : (kt + 1) * 128, :]
                    )
                for kt in range(W2A):
                    nc.sync.dma_start(
                        out=w2_t[kt], in_=d_w2[kt * 128 : (kt + 1) * 128, :]
                    )

                # ones column of V: 12 cheap strided copies from the dense
                # tok-major validity tile (a direct strided DMA generates
                # 2-byte-packet descriptor spam and a ~26us drain)
                for h in range(HEADS):
                    nc.vector.tensor_copy(
                        sb_V[:, :, h, DK : DK + 1],
                        sb_onescT[:].rearrange("p (t o) -> p t o", o=1),
                    )

                # Q^T feat-major, kt-outer so compute starts on chunk 0
                with tc.tile_pool(name="ps_q", bufs=1, space="PSUM") as ps_q:
                    psq = [
                        ps_q.tile([128, BLK], f32, tag=f"q{ft}", name=f"psq{ft}")
                        for ft in range(FT)
                    ]
                    for kt in range(FT):
                        for ft in range(FT):
                            nc.tensor.matmul(
                                psq[ft],
                                w_q[:, kt, ft * 128 : (ft + 1) * 128],
                                sb_xTb[:, kt, :],
                                start=(kt == 0),
                                stop=(kt == FT - 1),
                            )
                    for ft in range(FT):
                        nc.scalar.activation(
                            sb_Q[:, ft, :], psq[ft], AF.Identity,
                            bias=sb_bqT[:, ft : ft + 1],
                        )

                # V tok-major over compacted keys, kt-outer in chunk-paced
                # passes of <=8 concurrent PSUM groups
                with tc.tile_pool(name="ps_v", bufs=1, space="PSUM") as ps_v:
                    for vpass in VPASS:
                        psv = [
                            ps_v.tile([128, 384], f32, tag=f"v{i}", name=f"psv{i}")
                            for i in range(len(vpass))
                        ]
                        for kt in range(FT):
                            for i, (nh, tt) in enumerate(vpass):
                                nc.tensor.matmul(
                                    psv[i],
                                    sb_xkvT[:, kt, tt * 128 : (tt + 1) * 128],
                                    w_v[:, kt, nh * 384 : (nh + 1) * 384],
                                    start=(kt == 0),
                                    stop=(not general and kt == FT - 1),
                                )
                        for i, (nh, tt) in enumerate(vpass):
                            if general:
                                nc.tensor.matmul(
                                    psv[i],
                                    sb_onesc[0:1, tt * 128 : (tt + 1) * 128],
                                    sb_bvrow[0:1, nh * 384 : (nh + 1) * 384],
                                    start=False,
                                    stop=True,
                                )
                            nc.vector.tensor_copy(
                                sb_V[:, tt, nh * 6 : (nh + 1) * 6, 0:DK],
                                psv[i][:].rearrange("p (h d) -> p h d", d=DK),
                            )

                with (
                    tc.tile_pool(name="ps1", bufs=2, space="PSUM") as ps1,
                    tc.tile_pool(name="ps_sc", bufs=2, space="PSUM") as ps_sc,
                    tc.tile_pool(name="ps_z", bufs=1, space="PSUM") as ps_z,
                ):
                    def k_proj(ft):
                        for c0, c1 in KCH:
                            ps = ps1.tile([128, c1 - c0], f32, tag="p", name="ps_k")
                            for kt in range(FT):
                                nc.tensor.matmul(
                                    ps,
                                    w_k[:, kt, ft * 128 : (ft + 1) * 128],
                                    sb_xkvT[:, kt, c0:c1],
                                    start=(kt == 0),
                                    stop=(not general and kt == FT - 1),
                                )
                            if general:
                                nc.tensor.matmul(
                                    ps,
                                    sb_bkrow[0:1, ft * 128 : (ft + 1) * 128],
                                    sb_onesc[0:1, c0:c1],
                                    start=False,
                                    stop=True,
                                )
                            nc.vector.tensor_copy(sb_K[:, ft, c0:c1], ps)

                    def scores(hp):
                        ets = ([], [])
                        for kt2 in range(STC):
                            ps = ps_sc.tile([128, 2 * BLK], f32, tag="sc", name="ps_s")
                            for half in (0, 1):
                                ho = half * 64
                                nc.tensor.matmul(
                                    ps[:, half * BLK : (half + 1) * BLK],
                                    sb_K[ho : ho + 64, hp, kt2 * 128 : (kt2 + 1) * 128],
                                    sb_Q[ho : ho + 64, hp, :],
                                    start=True,
                                    stop=True,
                                )
                            et = expp.tile([128, 2 * BLK], bf16, tag="exp", name="et")
                            nc.scalar.activation(et, ps, AF.Exp, scale=ISCALE)
                            for half in (0, 1):
                                ets[half].append(et[:, half * BLK : (half + 1) * BLK])
                        return ets

                    def attn_v(hp, ets):
                        # per-half: matmuls then immediately the softmax tail,
                        # so half 0's tail overlaps half 1's matmuls
                        for half in (0, 1):
                            h = 2 * hp + half
                            ho = half * 64
                            zp = ps_z.tile([DK + 1, BLK], f32, tag=f"z{half}",
                                           name="ps_z")
                            # seed with the masked-keys correction row
                            nc.tensor.matmul(
                                zp,
                                sb_crow[0:1, h * (DK + 1) : (h + 1) * (DK + 1)],
                                ones512b[:],
                                start=True,
                                stop=False,
                            )
                            for kt2 in range(STC):
                                nc.tensor.matmul(
                                    zp,
                                    sb_V[:, kt2, h, :],
                                    ets[half][kt2],
                                    start=False,
                                    stop=(kt2 == STC - 1),
                                )
                            # denominators are huge sums (>= 1): the ~18-bit fast
                            # reciprocal is ~5x cheaper and plenty exact.
                            # (bitwise-trick op: input must be in SBUF, not PSUM)
                            zrow = attsm.tile([1, BLK], f32, tag="zrow", name="zrow")
                            nc.vector.tensor_copy(zrow, zp[DK : DK + 1, :])
                            rsum = attsm.tile([1, BLK], f32, tag="rsum", name="rsum")
                            nc.vector.reciprocal_approx_fast(rsum, zrow)
                            # fan the [1,512] reciprocal out to 64 partitions on
                            # the (otherwise idle) GpSimd engine — no PE matmul,
                            # no PSUM bank, no DVE copy
                            rb = attsm.tile([64, BLK], f32, tag="rbs", name="rb")
                            nc.gpsimd.partition_broadcast(rb[:], rsum[:])
                            nc.vector.tensor_mul(
                                sb_zT[hp][ho : ho + 64, :], zp[0:DK, :], rb
                            )

                    k_proj(0)
                    ets_prev = scores(0)
                    for hp in range(1, HEADS // 2):
                        k_proj(hp)
                        attn_v(hp - 1, ets_prev)
                        ets_prev = scores(hp)
                    attn_v(HEADS // 2 - 1, ets_prev)

            kqv_cm.__exit__(None, None, None)

            # broadcast-row constants + the rest of w2, loaded into the space
            # the attention residents just freed
            const2_cm = tc.tile_pool(name="const2", bufs=1)
            const2 = const2_cm.__enter__()
            bo_bc = const2.tile([128, DIM], bf16)
            nc.gpsimd.dma_start(out=bo_bc, in_=bcast_ap(d_bo))
            b2_bc = const2.tile([128, DIM], bf16)
            nc.gpsimd.dma_start(out=b2_bc, in_=bcast_ap(d_b2))
            g1_bc = const2.tile([128, DIM], bf16)
            nc.gpsimd.dma_start(out=g1_bc, in_=bcast_ap(d_g1))
            bb1_bc = const2.tile([128, DIM], bf16)
            nc.gpsimd.dma_start(out=bb1_bc, in_=bcast_ap(d_bb1))
            g2_bc = const2.tile([128, DIM], bf16)
            nc.gpsimd.dma_start(out=g2_bc, in_=bcast_ap(d_g2))
            bb2_bc = const2.tile([128, DIM], bf16)
            nc.gpsimd.dma_start(out=bb2_bc, in_=bcast_ap(d_bb2))
            ident = const2.tile([128, 128], bf16)
            make_identity(nc, ident[:])

            w2b_cm = tc.tile_pool(name="w2b", bufs=1)
            w2b = w2b_cm.__enter__()
            for kt in range(W2A, HT):
                w2_kt = w2b.tile([128, DIM], bf16, tag=f"w2_{kt}", name=f"w2_{kt}")
                nc.sync.dma_start(out=w2_kt, in_=d_w2[kt * 128 : (kt + 1) * 128, :])
                w2_t.append(w2_kt)

            # ===== Phases 3-5 merged: per-token-tile pipeline =====
            # s (the row-sum of x) comes from accum_out on the producer's
            # bias-add, saving a [128,768] reduce on the LN critical path
            def ln_stats(x_ap, s, pool):
                junk = pool.tile([128, DIM], bf16, tag="ln_j")
                ssq = pool.tile([128, 1], f32, tag="ln_q")
                # x^2 row-sums on the ACT engine (accum_out) — keeps the
                # [128,768] pass off the DVE queue, which is the block
                # bottleneck
                nc.scalar.activation(junk, x_ap, AF.Square, accum_out=ssq)
                negmean = pool.tile([128, 1], f32, tag="ln_m")
                nc.vector.tensor_scalar_mul(negmean, s, -1.0 / DIM)
                m2 = pool.tile([128, 1], f32, tag="ln_m2")
                nc.vector.tensor_mul(m2, negmean, negmean)
                return ssq, negmean, m2

            def ln_apply(out_ap, x_ap, g_bc_t, resid_ap, pool, ssq, negmean,
                         m2):
                # var = E[x^2] - mean^2
                var = pool.tile([128, 1], f32, tag="ln_v")
                nc.vector.scalar_tensor_tensor(
                    out=var, in0=ssq, scalar=1.0 / DIM, in1=m2,
                    op0=ALU.mult, op1=ALU.subtract,
                )
                sd = pool.tile([128, 1], f32, tag="ln_sd")
                nc.scalar.activation(sd, var, AF.Sqrt, bias=eps_t[:])
                rstd = pool.tile([128, 1], f32, tag="ln_r")
                nc.vector.reciprocal(rstd, sd)
                nmr = pool.tile([128, 1], f32, tag="ln_nm")
                nc.vector.tensor_mul(nmr, negmean, rstd)
                if not general:
                    # g==1, b==0: normalized = (x*rstd)+nmr as ONE DVE
                    # tensor_scalar with two per-partition scalars — keeps
                    # the chain on one engine (each cross-engine sem hop
                    # measures 0.3-0.8us)
                    t1 = pool.tile([128, DIM], f32, tag="ln_t")
                    nc.vector.tensor_scalar(
                        out=t1, in0=x_ap, scalar1=rstd[:], scalar2=nmr[:],
                        op0=ALU.mult, op1=ALU.add,
                    )
                    nc.vector.tensor_add(out_ap, t1, resid_ap)
                    return
                # x*rstd on ACT; then (x*rstd - mu*rstd) * g fused on DVE
                nrm = pool.tile([128, DIM], f32, tag="ln_t")
                nc.scalar.mul(nrm, x_ap, rstd[:])
                tg = pool.tile([128, DIM], f32, tag="ln_tg")
                nc.vector.scalar_tensor_tensor(
                    out=tg, in0=nrm, scalar=nmr[:], in1=g_bc_t,
                    op0=ALU.add, op1=ALU.mult,
                )
                nc.vector.tensor_add(out_ap, tg, resid_ap)

            def layer_norm_to(out_ap, x_ap, g_bc_t, resid_ap, pool, s):
                ssq, negmean, m2 = ln_stats(x_ap, s, pool)
                ln_apply(out_ap, x_ap, g_bc_t, resid_ap, pool, ssq, negmean,
                         m2)

            # all four O-projs + LN1s first: LN1 latency hides under them.
            # kt 0..4 accumulate two token-tiles ahead; the kt=5 finisher
            # (which depends on the last head-pair's softmax tail) trails,
            # so the PE never waits on that tail.
            with (
                tc.tile_pool(name="ln1p", bufs=4) as ln1p,
                tc.tile_pool(name="ps_o", bufs=2, space="PSUM") as ps_o,
            ):
                ops = {}

                def o_start(tt):
                    prs = []
                    for nh in range(2):
                        ps = ps_o.tile([128, 384], f32, tag=f"o{nh}",
                                       name=f"ps_o{nh}")
                        for kt in range(FT - 1):
                            nc.tensor.matmul(
                                ps,
                                sb_zT[kt][:, tt * 128 : (tt + 1) * 128],
                                w_o[:, kt, nh * 384 : (nh + 1) * 384],
                                start=(kt == 0),
                                stop=False,
                            )
                        prs.append(ps)
                    ops[tt] = prs

                pres = {}

                def o_finish(tt):
                    # kt=5 finishers + bias-add STTs only; the LN chains are
                    # issued after ALL of these so their ACT<->DVE ping-pong
                    # latency never blocks the later STTs at the DVE queue
                    # head (which would stall the O PSUM rotation)
                    prs = ops.pop(tt)
                    l1pre = ln1p.tile([128, DIM], f32, tag="pre")
                    accs = []
                    for nh in range(2):
                        nc.tensor.matmul(
                            prs[nh],
                            sb_zT[FT - 1][:, tt * 128 : (tt + 1) * 128],
                            w_o[:, FT - 1, nh * 384 : (nh + 1) * 384],
                            start=False,
                            stop=True,
                        )
                        acc = ln1p.tile([128, 1], f32, tag=f"acc{nh}",
                                        name=f"acc{nh}")
                        nc.vector.scalar_tensor_tensor(
                            out=l1pre[:, nh * 384 : (nh + 1) * 384],
                            in0=prs[nh],
                            scalar=1.0,
                            in1=bo_bc[:, nh * 384 : (nh + 1) * 384],
                            op0=ALU.mult,
                            op1=ALU.add,
                            accum_out=acc,
                        )
                        accs.append(acc)
                    s = ln1p.tile([128, 1], f32, tag="ln_s")
                    nc.vector.tensor_add(s, accs[0], accs[1])
                    pres[tt] = (l1pre, ln_stats(l1pre[:], s, ln1p))

                o_start(0)
                o_start(1)
                o_finish(0)
                o_start(2)
                o_finish(1)
                o_start(3)
                o_finish(2)
                o_finish(3)
                for tt in range(TT):
                    l1pre, stats = pres.pop(tt)
                    if general:
                        xb1 = ln1p.tile([128, DIM], f32, tag="resid")
                        nc.vector.tensor_add(xb1, sb_xblk[:, tt, :], bb1_bc)
                        resid = xb1[:]
                    else:
                        resid = sb_xblk[:, tt, :]
                    ln_apply(sb_l1[:, tt, :], l1pre[:], g1_bc, resid, ln1p,
                             *stats)

            # per token tile: transpose -> FFN1 -> FFN2 -> LN2 -> store
            with (
                tc.tile_pool(name="ln2p", bufs=4) as ln2p,
                tc.tile_pool(name="l1tp", bufs=1) as l1tp,
                tc.tile_pool(name="hTp", bufs=1) as hTp,
                tc.tile_pool(name="outp", bufs=2) as outp,
                tc.tile_pool(name="ps_t", bufs=3, space="PSUM") as ps_t,
                tc.tile_pool(name="ps_f1", bufs=3, space="PSUM") as ps_f1,
                tc.tile_pool(name="ps_f2", bufs=2, space="PSUM") as ps_f2,
            ):
                out_r = d_out[:].rearrange("(t p) d -> p t d", p=128)
                # full-width l1^T, transposed per token tile as its LN1
                # lands; FFN1 then runs 24 N=512 groups (the ~200ns
                # per-group boundary tax made 96 N=128 groups cost ~14us
                # more)
                l1T = l1tp.tile([128, FT, BLK], bf16, tag="l1T", name="l1T")
                for tt in range(TT):
                    for ft in range(FT):
                        pst = ps_t.tile([128, 128], bf16, tag="tp")
                        nc.tensor.transpose(
                            pst, sb_l1[:, tt, ft * 128 : (ft + 1) * 128], ident[:]
                        )
                        ceng = nc.vector if ft % 2 else nc.scalar
                        if ceng is nc.vector:
                            nc.vector.tensor_copy(
                                l1T[:, ft, tt * 128 : (tt + 1) * 128], pst
                            )
                        else:
                            nc.scalar.copy(
                                l1T[:, ft, tt * 128 : (tt + 1) * 128], pst
                            )
                hT = hTp.tile([128, HT, BLK], bf16, tag="hT", name="hT")
                for ht2 in range(HT):
                    ps = ps_f1.tile([128, BLK], f32, tag="f1")
                    for kt in range(FT):
                        nc.tensor.matmul(
                            ps,
                            w1_t[kt][:, ht2 * 128 : (ht2 + 1) * 128],
                            l1T[:, kt, :],
                            start=(kt == 0),
                            stop=(kt == FT - 1),
                        )
                    # relu(x + b1) on ACT (per-partition bias)
                    nc.scalar.activation(
                        hT[:, ht2, :], ps, AF.Relu,
                        bias=sb_b1[:, ht2 : ht2 + 1],
                    )
                for tt in range(TT):
                    f2pre = ln2p.tile([128, DIM], f32, tag="pre")
                    accs = []
                    for nh in range(2):
                        ps = ps_f2.tile([128, 384], f32, tag="f2")
                        for kt in range(HT):
                            nc.tensor.matmul(
                                ps,
                                hT[:, kt, tt * 128 : (tt + 1) * 128],
                                w2_t[kt][:, nh * 384 : (nh + 1) * 384],
                                start=(kt == 0),
                                stop=(kt == HT - 1),
                            )
                        acc = ln2p.tile([128, 1], f32, tag=f"acc{nh}",
                                        name=f"acc{nh}")
                        nc.vector.scalar_tensor_tensor(
                            out=f2pre[:, nh * 384 : (nh + 1) * 384],
                            in0=ps,
                            scalar=1.0,
                            in1=b2_bc[:, nh * 384 : (nh + 1) * 384],
                            op0=ALU.mult,
                            op1=ALU.add,
                            accum_out=acc,
                        )
                        accs.append(acc)
                    s = ln2p.tile([128, 1], f32, tag="ln_s")
                    nc.vector.tensor_add(s, accs[0], accs[1])
                    if general:
                        l1b = ln2p.tile([128, DIM], f32, tag="resid")
                        nc.vector.tensor_add(l1b, sb_l1[:, tt, :], bb2_bc)
                        resid = l1b[:]
                    else:
                        resid = sb_l1[:, tt, :]
                    o_sb = outp.tile([128, DIM], f32, tag="osb")
                    layer_norm_to(o_sb[:], f2pre[:], g2_bc, resid, ln2p, s)
                    nc.sync.dma_start(out=out_r[:, tt, :], in_=o_sb)

            w2b_cm.__exit__(None, None, None)
            const2_cm.__exit__(None, None, None)
            zT_cm.__exit__(None, None, None)
            wpre_cm.__exit__(None, None, None)

    return nc


def _get_nc(general: bool = False, finalized=True):
    key = f"nc{int(general)}"
    if key not in _CACHE:
        _CACHE[key] = _build_program(general)
    nc = _CACHE[key]
    if finalized and not nc.is_finalized():
        nc.finalize()
    return nc


def make_in_maps(inputs: dict) -> list:
    x = np.asarray(inputs["x_n"], np.float32).reshape(B, S, DIM)
    mask = np.asarray(inputs["mask"]).reshape(B, S)
    w = {
        k: np.ascontiguousarray(np.asarray(inputs[k], np.float32).astype(BF16))
        for k in ("wq", "wk", "wv", "wo", "w1", "w2")
    }
    vecs = {
        "bo": inputs["bo"], "b1": inputs["b1"], "b2": inputs["b2"],
        "g1": inputs["ln1_g"], "bb1": inputs["ln1_b"],
        "g2": inputs["ln2_g"], "bb2": inputs["ln2_b"],
        "bqT": inputs["bq"],
    }
    vecs = {k: np.ascontiguousarray(np.asarray(v, np.float32)) for k, v in vecs.items()}
    brows = {
        "bkrow": np.asarray(inputs["bk"], np.float32).astype(BF16),
        "bvrow": np.asarray(inputs["bv"], np.float32).astype(BF16),
    }

    # per-batch compaction + masked-keys correction
    per_batch = []
    for b in range(B):
        mb = mask[b] != 0
        idx = np.nonzero(mb)[0]
        n_u = len(idx)
        if n_u > KC:
            raise RuntimeError(
                f"unmasked key count {n_u} exceeds compiled capacity {KC}"
            )
        xkv = np.zeros((KC, DIM), np.float32)
        xkv[:n_u] = x[b][idx]
        xkvT = np.ascontiguousarray(xkv.T.astype(BF16))
        onesc = np.zeros(KC, np.float32)
        onesc[:n_u] = 1.0
        msum = x[b][~mb].astype(np.float64).sum(axis=0)
        mcount = float((~mb).sum())
        wv64 = np.asarray(inputs["wv"], np.float64)
        bv64 = np.asarray(inputs["bv"], np.float64)
        cvec = (msum @ wv64 + mcount * bv64).astype(np.float32)  # [DIM]
        crow = np.zeros(HEADS * (DK + 1), np.float32)
        ch = cvec.reshape(HEADS, DK)
        for h in range(HEADS):
            crow[h * (DK + 1) : h * (DK + 1) + DK] = ch[h]
            crow[h * (DK + 1) + DK] = mcount
        per_batch.append(
            {"xkvT": xkvT, "onesc": onesc.astype(BF16), "crow": crow.astype(BF16)}
        )

    in_maps = []
    for c in range(N_CORES):
        b, blk = c // NBLK, c % NBLK
        xb = x[b]
        xblk = np.ascontiguousarray(xb[blk * BLK : (blk + 1) * BLK])
        xTb = np.ascontiguousarray(xblk.T.astype(BF16))
        m = {"xTb": xTb, "xb": xblk}
        m.update(per_batch[b])
        m.update(w)
        m.update(vecs)
        m.update(brows)
        in_maps.append(m)
    return in_maps


def assemble(per_core_out: list) -> np.ndarray:
    blocks = [np.asarray(o, np.float32) for o in per_core_out]
    full = np.concatenate(blocks, axis=0).reshape(B, S, DIM)
    return full


def kernel(**inputs) -> np.ndarray:
    from concourse.bass_utils import run_bass_kernel_spmd

    general = bool(
        np.any(np.asarray(inputs["bk"]))
        or np.any(np.asarray(inputs["bv"]))
        or np.any(np.asarray(inputs["ln1_b"]))
        or np.any(np.asarray(inputs["ln2_b"]))
        or np.any(np.asarray(inputs["ln1_g"]) != 1)
        or np.any(np.asarray(inputs["ln2_g"]) != 1)
    )
    nc = _get_nc(general)
    in_maps = make_in_maps(inputs)
    res = run_bass_kernel_spmd(nc, in_maps, list(range(N_CORES)))
    return assemble([r["out"] for r in res.results])
